# revision 1
# baseline (speedup 1.0000x reference)
"""AME-ODE (mixture-of-experts ODE, RK4) Trainium2 kernel.

Strategy: pure data parallel over batch. Each of the 8 NeuronCores
integrates B/8 = 2048 trajectories; the tiny expert/gating parameters are
replicated (packed host-side into matmul-ready layouts). No collectives.

On-chip layout: state x is kept transposed [D=32 partitions, B=2048 free]
in float32r (TF32-like storage, 1 col/cycle matmuls). Work is split into
two 1024-column chunks that the Tile scheduler runs antiphased so the
TensorE / ScalarE / VectorE phases of one chunk fill the other's gaps.

Per dyn() eval (36 = 9 RK4 steps x 4 stages), per chunk:
  - expert pairs q (4 pairs of 2 experts, outputs stacked on partitions):
      mm1[128,512]x2 = W1pair[32,128]^T @ x_stage   (float32r, PSUM)
      tanh1 -> h1  (ScalarE, bias = t*W1_trow + b1, host-precomputed/eval)
      mm2 = blockdiag(W2 pair)[128,128]^T @ h1      (float32r)
      tanh2 -> h2  (bf16)
  - dxinit = mean_e f_e: PSUM-accumulate vstack(W3 pair)[128,32]^T @ h2_q
    over pairs (the b3 term folds into the evacuation op; skipped if 0)
  - gating (cat-free): g = tanh(Gw1x^T x + Gw1d^T dxinit + Gb1); then
    per-128 b-tile logitsT[128,8] = g_tile[64,128]^T @ Gw2 (bf16) puts
    batch on partitions so softmax reduces along the free axis:
    exp -> reduce_sum -> reciprocal -> normalized wgtT[128, 8e x 8tiles]
  - transpose wgtT tiles back to wide wgt[8,1024] (TensorE transposes)
  - combine: B'_q[128,512] = S_q^T @ wgt broadcasts weights across the
    pair's feature partitions; h2w = h2 * B' (VectorE); PSUM-accumulate
    k += vstack(W3 pair)^T @ h2w. This computes sum_e w_e * f_e without
    ever materializing f. k lands in a 1-bank [64,512] PSUM tile per
    chunk (column halves folded onto partitions).
RK4 stage/accumulator updates are scalar_tensor_tensor ops reading k
straight from PSUM; the trajectory is written out per step via TensorE
transposes back to batch-major rows.

PSUM budget (8 banks): mm1 2x[128,512] + mm2 2x[128,512] (rotating tanh
sources), 2 misc 1-bank slots (dx/gating/logits/wgt-transpose/broadcast),
and one [64,512] k-tile per chunk.
"""

import os
import sys

import numpy as np

if "/opt/trn_rl_repo" not in sys.path:
    sys.path.insert(0, "/opt/trn_rl_repo")

import concourse.mybir as mybir
from concourse import bacc, bass
from concourse.bass_utils import run_bass_kernel_spmd
from concourse.tile import TileContext

F32 = mybir.dt.float32
F32R = mybir.dt.float32r
BF16 = mybir.dt.bfloat16
AF = mybir.ActivationFunctionType
ALU = mybir.AluOpType
AX = mybir.AxisListType

B, D, E, H, T = 16384, 32, 8, 64, 10
NCORES = 8
BC = B // NCORES          # 2048 batch per core
CHUNK = 1024              # column chunk (PSUM sizing)
NCH = BC // CHUNK         # 2
NPAIR = E // 2            # 4 expert pairs
NT = CHUNK // 128         # 8 b-tiles of 128 per chunk

NEV = 4 * (T - 1)  # 36 dyn evals


def _eval_ts(ts):
    out = []
    for s in range(T - 1):
        t0, t1 = ts[s], ts[s + 1]
        dt = t1 - t0
        out += [t0, t0 + dt / 2.0, t0 + dt / 2.0, t1]
    return out


# exec time of the last hardware run (ns), for test harnesses
LAST_EXEC_NS = None
LAST_RESULT = None


def _r(x):
    return x.bitcast(F32R)


def _build(t_span: np.ndarray, use_b3: bool, use_gb2: bool):
    nc = bacc.Bacc(None)

    x0c = nc.declare_dram_parameter("x0c", [BC, D], F32, isOutput=False)
    cW1P = nc.declare_dram_parameter("cW1P", [D, NPAIR * 128], F32R, isOutput=False)
    cb1t = nc.declare_dram_parameter("cb1t", [128, NPAIR * NEV], F32, isOutput=False)
    cW2b = nc.declare_dram_parameter("cW2b", [128, NPAIR * 128], F32R, isOutput=False)
    cb2p = nc.declare_dram_parameter("cb2p", [128, NPAIR], F32, isOutput=False)
    cW3s = nc.declare_dram_parameter("cW3s", [128, NPAIR * D], BF16, isOutput=False)
    cb3m = nc.declare_dram_parameter("cb3m", [D, 1], F32, isOutput=False)
    cb3b = nc.declare_dram_parameter("cb3b", [E, D], BF16, isOutput=False)
    cGw1x = nc.declare_dram_parameter("cGw1x", [D, H], F32R, isOutput=False)
    cGw1d = nc.declare_dram_parameter("cGw1d", [D, H], F32R, isOutput=False)
    cGb1 = nc.declare_dram_parameter("cGb1", [H, 1], F32, isOutput=False)
    cGw2 = nc.declare_dram_parameter("cGw2", [H, E], BF16, isOutput=False)
    cSal = nc.declare_dram_parameter("cSal", [E, NPAIR * 128], BF16, isOutput=False)
    cGb2T = nc.declare_dram_parameter("cGb2T", [128, NT * E], F32, isOutput=False)
    cIde = nc.declare_dram_parameter("cIde", [128, 128], F32, isOutput=False)
    cIdeR = nc.declare_dram_parameter("cIdeR", [128, 128], F32R, isOutput=False)
    out = nc.declare_dram_parameter("out", [BC, T, D], F32, isOutput=True)

    ts = [float(v) for v in t_span]

    with TileContext(nc) as tc:
        with (
            tc.tile_pool(name="const", bufs=1) as cp,
            tc.tile_pool(name="state", bufs=2) as sp,
            tc.tile_pool(name="xsp", bufs=4) as xsp,
            tc.tile_pool(name="dxp", bufs=3) as dxp,
            tc.tile_pool(name="h1p", bufs=4) as h1p,
            tc.tile_pool(name="h2p", bufs=12) as h2p,
            tc.tile_pool(name="h2wp", bufs=3) as h2wp,
            tc.tile_pool(name="bimg", bufs=4) as bip,
            tc.tile_pool(name="gate", bufs=3) as gp,
            tc.tile_pool(name="small", bufs=4) as smp,
            tc.tile_pool(name="stage", bufs=3) as stp,
            tc.tile_pool(name="psA", bufs=1, space="PSUM") as psA,
            tc.tile_pool(name="psB", bufs=2, space="PSUM") as psB,
            tc.tile_pool(name="psK", bufs=1, space="PSUM") as psK,
        ):
            # ---- load constants ----
            W1P = cp.tile([D, NPAIR * 128], F32R)
            B1T = cp.tile([128, NPAIR * NEV], F32)
            W2b = cp.tile([128, NPAIR * 128], F32R)
            b2p = cp.tile([128, NPAIR], F32)
            W3s = cp.tile([128, NPAIR * D], BF16)
            b3m = cp.tile([D, 1], F32)
            b3b = cp.tile([E, D], BF16)
            Gw1x = cp.tile([D, H], F32R)
            Gw1d = cp.tile([D, H], F32R)
            Gb1 = cp.tile([H, 1], F32)
            Gw2 = cp.tile([H, E], BF16)
            Sal = cp.tile([E, NPAIR * 128], BF16)
            Gb2T = cp.tile([128, NT * E], F32)
            Ide = cp.tile([128, 128], F32)
            IdeR = cp.tile([128, 128], F32R)
            for tile, par in [
                (W1P, cW1P), (B1T, cb1t), (W2b, cW2b),
                (b2p, cb2p), (W3s, cW3s), (b3m, cb3m), (b3b, cb3b),
                (Gw1x, cGw1x), (Gw1d, cGw1d), (Gb1, cGb1), (Gw2, cGw2),
                (Sal, cSal), (Gb2T, cGb2T), (Ide, cIde), (IdeR, cIdeR),
            ]:
                nc.sync.dma_start(out=tile[:, :], in_=par[:, :])

            # ---- load x0 and emit t=0 output ----
            xrm = sp.tile([128, (BC // 128) * D], F32)  # [128, 16*32] row-major
            nc.sync.dma_start(
                out=xrm[:, :].rearrange("p (n d) -> p n d", d=D),
                in_=x0c.rearrange("(n p) d -> p n d", p=128),
            )
            nc.sync.dma_start(
                out=out[:, 0, :].rearrange("(n p) d -> p n d", p=128),
                in_=xrm[:, :].rearrange("p (n d) -> p n d", d=D),
            )
            x = sp.tile([D, BC], F32R)
            for i in range(BC // 128):
                pt = psB.tile([D, 128], F32, tag="misc")
                nc.tensor.transpose(
                    pt[0:D, 0:128], xrm[:, i * D:(i + 1) * D], Ide[:, :]
                )
                nc.vector.tensor_copy(x[:, i * 128:(i + 1) * 128], pt[0:D, 0:128])

            # ---------------- dyn() ----------------
            def dyn(xins, ev, kaccs):
                """xins: per-chunk [D, CHUNK] state APs. Writes the weighted
                dynamics k into kaccs[ch][0:D, :]."""
                b1t = B1T[:, ev * NPAIR:(ev + 1) * NPAIR]
                for ch in range(NCH):
                    xin = xins[ch]
                    h2s = []
                    for q in range(NPAIR):
                        h1 = h1p.tile([128, CHUNK], F32R, tag="h1")
                        for c in range(CHUNK // 512):
                            ph1 = psA.tile([128, 512], F32, tag="mm", name="ph1", bufs=2)
                            nc.tensor.matmul(
                                ph1[:, :],
                                W1P[:, q * 128:(q + 1) * 128],
                                xin[:, c * 512:(c + 1) * 512],
                                start=True, stop=True,
                            )
                            nc.scalar.activation(
                                h1[:, c * 512:(c + 1) * 512], ph1[:, :], AF.Tanh,
                                bias=b1t[:, q:q + 1], scale=1.0,
                            )
                        h2 = h2p.tile([128, CHUNK], BF16, tag="h2")
                        for c in range(CHUNK // 512):
                            ph2 = psA.tile([128, 512], F32, tag="mm2", name="ph2", bufs=2)
                            nc.tensor.matmul(
                                ph2[:, :],
                                W2b[:, q * 128:(q + 1) * 128],
                                h1[:, c * 512:(c + 1) * 512],
                                start=True, stop=True,
                            )
                            nc.scalar.activation(
                                h2[:, c * 512:(c + 1) * 512], ph2[:, :], AF.Tanh,
                                bias=b2p[:, q:q + 1], scale=1.0,
                            )
                        h2s.append(h2)
                    # dxinit = mean_e f_e (linear part), deferred accumulation
                    dxs = dxp.tile([D, CHUNK], F32R, tag="dxs")
                    for c in range(CHUNK // 512):
                        pdx = psB.tile([D, 512], F32, tag="misc", name="pdx")
                        for q in range(NPAIR):
                            nc.tensor.matmul(
                                pdx[:, :],
                                W3s[:, q * D:(q + 1) * D],
                                h2s[q][:, c * 512:(c + 1) * 512],
                                start=(q == 0), stop=(q == NPAIR - 1),
                            )
                        nc.vector.tensor_scalar(
                            out=dxs[:, c * 512:(c + 1) * 512], in0=pdx[:, :],
                            scalar1=1.0 / E, scalar2=b3m[:, 0:1],
                            op0=ALU.mult, op1=ALU.add,
                        )
                    # gating: g = tanh(Gw1x^T x + Gw1d^T dxinit + Gb1)
                    g = gp.tile([H, CHUNK], BF16, tag="g")
                    for c in range(CHUNK // 512):
                        pg = psB.tile([H, 512], F32, tag="misc", name="pg")
                        nc.tensor.matmul(
                            pg[:, :], Gw1x[:, :],
                            xin[:, c * 512:(c + 1) * 512],
                            start=True, stop=False,
                        )
                        nc.tensor.matmul(
                            pg[:, :], Gw1d[:, :],
                            dxs[:, c * 512:(c + 1) * 512],
                            start=False, stop=True,
                        )
                        nc.scalar.activation(
                            g[:, c * 512:(c + 1) * 512], pg[:, :], AF.Tanh,
                            bias=Gb1[:, 0:1], scale=1.0,
                        )
                    plog = psB.tile([128, NT * E], F32, tag="misc")
                    for i in range(NT):
                        nc.tensor.matmul(
                            plog[:, i * E:(i + 1) * E],
                            g[:, i * 128:(i + 1) * 128],
                            Gw2[:, :],
                            start=True, stop=True,
                        )
                    if use_gb2:
                        plogc = smp.tile([128, NT * E], F32, tag="plogc")
                        nc.vector.tensor_add(plogc[:, :], plog[:, :], Gb2T[:, :])
                        exp_in = plogc[:, :]
                    else:
                        exp_in = plog[:, :]
                    expT = smp.tile([128, NT * E], F32, tag="expT")
                    nc.scalar.activation(
                        expT[:, :], exp_in, AF.Exp, bias=0.0, scale=1.0
                    )
                    den = smp.tile([128, NT], F32, tag="den")
                    nc.vector.reduce_sum(
                        out=den[:, :],
                        in_=expT[:, :].rearrange("p (n e) -> p n e", e=E),
                        axis=AX.X,
                    )
                    rec = smp.tile([128, NT], F32, tag="rec")
                    nc.vector.reciprocal(rec[:, :], den[:, :])
                    wgtT = smp.tile([128, NT * E], F32, tag="wgtT")
                    nc.vector.tensor_mul(
                        wgtT[:, :].rearrange("p (n e) -> p n e", e=E),
                        expT[:, :].rearrange("p (n e) -> p n e", e=E),
                        rec[:, :].to_broadcast((128, NT, E)),
                    )
                    # transpose to wide [E, CHUNK]
                    W8 = gp.tile([E, CHUNK], BF16, tag="w8")
                    for h in range(2):
                        pW8 = psB.tile([E, 512], F32, tag="misc", name="pW8")
                        for i in range(NT // 2):
                            nc.tensor.transpose(
                                pW8[:, i * 128:(i + 1) * 128],
                                wgtT[:, (h * 4 + i) * E:(h * 4 + i + 1) * E],
                                Ide[:, :],
                            )
                        nc.vector.tensor_copy(
                            W8[:, h * 512:(h + 1) * 512], pW8[:, :]
                        )
                    # combine: k = sum_e w_e f_e accumulated into kacc;
                    # column half c lives on partitions [c*D, (c+1)*D)
                    kacc = kaccs[ch]
                    for q in range(NPAIR):
                        h2w = h2wp.tile([128, CHUNK], BF16, tag="h2w")
                        for c in range(CHUNK // 512):
                            pB = psB.tile([128, 512], F32, tag="misc", name="pB")
                            nc.tensor.matmul(
                                pB[:, :],
                                Sal[:, q * 128:(q + 1) * 128],
                                W8[:, c * 512:(c + 1) * 512],
                                start=True, stop=True,
                            )
                            nc.vector.tensor_mul(
                                h2w[:, c * 512:(c + 1) * 512],
                                h2s[q][:, c * 512:(c + 1) * 512],
                                pB[:, :],
                            )
                            nc.tensor.matmul(
                                kacc[c * D:(c + 1) * D, :],
                                W3s[:, q * D:(q + 1) * D],
                                h2w[:, c * 512:(c + 1) * 512],
                                start=(q == 0),
                                stop=(not use_b3 and q == NPAIR - 1),
                                skip_group_check=True,
                            )
                    if use_b3:
                        for c in range(CHUNK // 512):
                            nc.tensor.matmul(
                                kacc[c * D:(c + 1) * D, :],
                                b3b[:, :],
                                W8[:, c * 512:(c + 1) * 512],
                                start=False, stop=True,
                                skip_group_check=True,
                            )

            # ---------------- RK4 loop ----------------
            def store_step(xt, step):
                stg = stp.tile([128, (BC // 128) * D], F32, tag="stg")
                pt = psB.tile([128, 512], F32, tag="misc")
                for i in range(BC // 128):
                    nc.tensor.transpose(
                        pt[:, i * D:(i + 1) * D].bitcast(F32R),
                        xt[:, i * 128:(i + 1) * 128],
                        IdeR[0:D, 0:D],
                    )
                nc.vector.tensor_copy(stg[:, :], pt[:, 0:(BC // 128) * D])
                nc.sync.dma_start(
                    out=out[:, step, :].rearrange("(n p) d -> p n d", p=128),
                    in_=stg[:, :].rearrange("p (n d) -> p n d", d=D),
                )

            def new_xs():
                return [xsp.tile([D, CHUNK], F32R, tag="xs", name="xs") for _ in range(NCH)]

            def k_stt(kac, ch, scalar, in1, out, in1_off=0, out_off=0):
                """out[:, c-half] = kac-half * scalar + in1[:, c-half]."""
                for c in range(CHUNK // 512):
                    nc.vector.scalar_tensor_tensor(
                        out=out[:, out_off + c * 512:out_off + (c + 1) * 512],
                        in0=kac[ch][c * D:(c + 1) * D, :],
                        scalar=scalar,
                        in1=in1[:, in1_off + c * 512:in1_off + (c + 1) * 512],
                        op0=ALU.mult, op1=ALU.add,
                    )

            for s in range(T - 1):
                t0, t1 = ts[s], ts[s + 1]
                dt = t1 - t0
                kac1 = [psK.tile([2 * D, 512], F32, tag="k0", name="k0"), psK.tile([2 * D, 512], F32, tag="k1", name="k1")]
                xins1 = [x[:, ch * CHUNK:(ch + 1) * CHUNK] for ch in range(NCH)]
                dyn(xins1, 4 * s + 0, kac1)

                acc = sp.tile([D, BC], F32, tag="acc_x")
                xs2 = new_xs()
                for ch in range(NCH):
                    off = ch * CHUNK
                    k_stt(kac1, ch, dt / 6.0, x, acc, in1_off=off, out_off=off)
                    k_stt(kac1, ch, dt / 2.0, x, xs2[ch], in1_off=off)
                kac2 = [psK.tile([2 * D, 512], F32, tag="k0", name="k0"), psK.tile([2 * D, 512], F32, tag="k1", name="k1")]
                dyn(xs2, 4 * s + 1, kac2)

                xs3 = new_xs()
                for ch in range(NCH):
                    off = ch * CHUNK
                    k_stt(kac2, ch, dt / 3.0, acc, acc, in1_off=off, out_off=off)
                    k_stt(kac2, ch, dt / 2.0, x, xs3[ch], in1_off=off)
                kac3 = [psK.tile([2 * D, 512], F32, tag="k0", name="k0"), psK.tile([2 * D, 512], F32, tag="k1", name="k1")]
                dyn(xs3, 4 * s + 2, kac3)

                xs4 = new_xs()
                for ch in range(NCH):
                    off = ch * CHUNK
                    k_stt(kac3, ch, dt / 3.0, acc, acc, in1_off=off, out_off=off)
                    k_stt(kac3, ch, dt, x, xs4[ch], in1_off=off)
                kac4 = [psK.tile([2 * D, 512], F32, tag="k0", name="k0"), psK.tile([2 * D, 512], F32, tag="k1", name="k1")]
                dyn(xs4, 4 * s + 3, kac4)

                xn = sp.tile([D, BC], F32R, tag="x_state")
                for ch in range(NCH):
                    off = ch * CHUNK
                    k_stt(kac4, ch, dt / 6.0, acc, xn, in1_off=off, out_off=off)
                x = xn
                store_step(x, s + 1)

    nc.finalize()
    return nc


_CACHE = {}


def _pack_inputs(x0, t_span, W1, b1, W2, b2, W3, b3, Gw1, Gb1, Gw2, Gb2):
    import ml_dtypes

    f32 = np.float32
    W1 = np.asarray(W1, f32)
    b1 = np.asarray(b1, f32)
    W2 = np.asarray(W2, f32)
    b2 = np.asarray(b2, f32)
    W3 = np.asarray(W3, f32)
    b3 = np.asarray(b3, f32)
    Gw1 = np.asarray(Gw1, f32)
    Gb1 = np.asarray(Gb1, f32)
    Gw2 = np.asarray(Gw2, f32)
    Gb2 = np.asarray(Gb2, f32)

    W1P = np.zeros((D, NPAIR * 128), f32)
    w1t = np.zeros((128, NPAIR), f32)
    b1p = np.zeros((128, NPAIR), f32)
    tl = _eval_ts([float(v) for v in np.asarray(_PACK_TSPAN, f32)])
    
    W2b = np.zeros((128, NPAIR * 128), f32)
    b2p = np.zeros((128, NPAIR), f32)
    W3s = np.zeros((128, NPAIR * D), f32)
    Sal = np.zeros((E, NPAIR * 128), f32)
    for q in range(NPAIR):
        e0, e1 = 2 * q, 2 * q + 1
        W1P[:, q * 128:q * 128 + 64] = W1[e0, :D, :]
        W1P[:, q * 128 + 64:(q + 1) * 128] = W1[e1, :D, :]
        w1t[0:64, q] = W1[e0, D, :]
        w1t[64:128, q] = W1[e1, D, :]
        b1p[0:64, q] = b1[e0]
        b1p[64:128, q] = b1[e1]
        W2b[0:64, q * 128:q * 128 + 64] = W2[e0]
        W2b[64:128, q * 128 + 64:(q + 1) * 128] = W2[e1]
        b2p[0:64, q] = b2[e0]
        b2p[64:128, q] = b2[e1]
        W3s[0:64, q * D:(q + 1) * D] = W3[e0]
        W3s[64:128, q * D:(q + 1) * D] = W3[e1]
        Sal[e0, q * 128:q * 128 + 64] = 1.0
        Sal[e1, q * 128 + 64:(q + 1) * 128] = 1.0
    b3m = (b3.mean(axis=0) / 1.0).reshape(D, 1).astype(f32)
    # note: dxinit = (sum_e W3_e^T h2_e)/E + mean_e(b3); the /E is applied in
    # tensor_scalar (scalar1=1/E) so b3m must be the already-averaged bias.
    Gb2T = np.tile(Gb2[None, :], (128, NT)).astype(f32)
    b1t_all = np.zeros((128, NPAIR * NEV), f32)
    for ev, tv in enumerate(tl):
        b1t_all[:, ev * NPAIR:(ev + 1) * NPAIR] = w1t * np.float32(tv) + b1p
    const = {
        "cW1P": W1P,
        "cb1t": b1t_all,
        "cW2b": W2b,
        "cb2p": b2p,
        "cW3s": W3s.astype(ml_dtypes.bfloat16),
        "cb3m": b3m,
        "cb3b": b3.astype(ml_dtypes.bfloat16),
        "cGw1x": Gw1[0:D, :],
        "cGw1d": Gw1[D:2 * D, :],
        "cGb1": Gb1.reshape(H, 1),
        "cGw2": Gw2.astype(ml_dtypes.bfloat16),
        "cSal": Sal.astype(ml_dtypes.bfloat16),
        "cGb2T": Gb2T,
        "cIde": np.eye(128, dtype=f32),
        "cIdeR": np.eye(128, dtype=f32),
    }
    x0 = np.asarray(x0, f32)
    in_maps = []
    for i in range(NCORES):
        m = dict(const)
        m["x0c"] = np.ascontiguousarray(x0[i * BC:(i + 1) * BC])
        in_maps.append(m)
    return in_maps


def kernel(x0, t_span, W1, b1, W2, b2, W3, b3, Gw1, Gb1, Gw2, Gb2, trace=False):
    global LAST_EXEC_NS, LAST_RESULT, _PACK_TSPAN
    _PACK_TSPAN = np.asarray(t_span, np.float32)
    t_span = np.asarray(t_span, np.float32)
    use_b3 = bool(np.any(np.asarray(b3)))
    use_gb2 = bool(np.any(np.asarray(Gb2)))
    key = (tuple(np.round(t_span, 7).tolist()), use_b3, use_gb2)
    if key not in _CACHE:
        _CACHE[key] = _build(t_span, use_b3, use_gb2)
    nc = _CACHE[key]
    in_maps = _pack_inputs(
        x0, t_span, W1, b1, W2, b2, W3, b3, Gw1, Gb1, Gw2, Gb2
    )
    res = run_bass_kernel_spmd(
        nc, in_maps, core_ids=list(range(NCORES)), trace=trace
    )
    LAST_EXEC_NS = res.exec_time_ns
    LAST_RESULT = res
    outs = [r["out"] for r in res.results]
    return np.concatenate(outs, axis=0)


def _pack_inputs_entry(inputs):
    global _PACK_TSPAN
    _PACK_TSPAN = np.asarray(inputs["t_span"], np.float32)
    return _pack_inputs(**inputs)

